# revision 41
# baseline (speedup 1.0000x reference)
"""Trainium2 Bass kernel for nn_Discriminator2 (bilinear discriminator scores).

Math: with hc0 = h_c[0] [N, D], W0 = W[0] [D, D]:
    v      = hc0 @ W0.T                      [N, D]   (tensor engine, bf16)
    sc1[n] = dot(h_pl[0][n], v[n]) + b       [N]      (fused DVE mult+reduce)
    sc2[s,n] = dot(hc0[sample[s,n]], v[n]) + b        (indirect-DMA gather + DVE)
    out    = [sc1 | sc2.flat | sc2.flat]     [1, N + 2*S*N]

Sharding: nodes (N) split evenly across 8 cores; hc0 replicated on every core
so gathers resolve locally; W replicated; h_pl / sample_list sharded by node.

All streamed data is bf16 (tolerance is 2e-2; measured pipeline error ~3.6e-3),
halving HBM traffic and SWDGE ring backpressure vs fp32. The five per-node dot
products per tile run as fused mult+reduce on DVE (scalar_tensor_tensor with
accum_out; first `stt_rows` rows) with the remainder as DVE bf16 mult +
ScalarE Copy-activation accum. Bias is one epilogue add over sc_acc.
NOTE: tensor_tensor_reduce (custom DVE uop) hangs this HW path — do not use.
"""

import sys

for _p in ("/opt/trn_rl_repo",):
    if _p not in sys.path:
        sys.path.insert(0, _p)

import ml_dtypes
import numpy as np

import concourse.bass as bass
import concourse.mybir as mybir
import concourse.tile as tile
from concourse import bacc
from concourse.bass_utils import run_bass_kernel_spmd

P = 128  # partitions
BF16 = ml_dtypes.bfloat16


class Cfg:
    """Problem geometry. Full-size defaults; shrink for CoreSim validation."""

    def __init__(self, n_table=100000, nodes_per_core=12500, d=512, s=4,
                 n_cores=8, super_tile=4, stt_rows=5, nq=4, gather_cols=1,
                 host_gather=False, pool_rows=0, layout_b=False):
        self.n_table = n_table          # rows of the gather table (full N)
        self.nodes_per_core = nodes_per_core
        self.d = d
        self.s = s
        self.n_cores = n_cores
        self.super_tile = super_tile    # node-tiles per hcT/hpl DMA block
        self.stt_rows = stt_rows        # rows via fused scalar_tensor_tensor
        self.nq = nq                    # SWDGE queues for gathers
        self.gather_cols = gather_cols  # index columns per indirect DMA call
        self.host_gather = host_gather  # hc0[sample] prepared on host, streamed
        self.pool_rows = pool_rows      # score rows fused on GpSimd (needs
                                        # host_gather to free the Q7)
        self.layout_b = layout_b        # d-on-partitions; PE one-hot reduces
        self.tiles = -(-nodes_per_core // P)        # ceil
        if layout_b:
            assert host_gather
            self.tiles = -(-self.tiles // super_tile) * super_tile
        self.npad = self.tiles * P
        self.kc = d // P                # contraction chunks
        self.n_super = self.tiles // super_tile
        self.fsz = super_tile * P       # nodes per super-block (layout_b)


FULL = Cfg(host_gather=True, layout_b=True)


def build_nc(cfg: Cfg):
    D, S, KC, TILES = cfg.d, cfg.s, cfg.kc, cfg.tiles
    bf16 = mybir.dt.bfloat16
    f32 = mybir.dt.float32

    nc = bacc.Bacc("TRN2", target_bir_lowering=False, debug=False,
                   num_swdge_queues=cfg.nq)
    if cfg.host_gather:
        gd = nc.dram_tensor("g", [cfg.npad, S * D], bf16,
                            kind="ExternalInput").ap()
    else:
        hc = nc.dram_tensor("hc", [cfg.n_table, D], bf16,
                            kind="ExternalInput").ap()
        idx = nc.dram_tensor("idx", [P, TILES * S], mybir.dt.int32,
                             kind="ExternalInput").ap()
    hcT = nc.dram_tensor("hcT", [D, cfg.npad], bf16, kind="ExternalInput").ap()
    hpl = nc.dram_tensor("hpl", [cfg.npad, D], bf16, kind="ExternalInput").ap()
    wt = nc.dram_tensor("wt", [D, D], bf16, kind="ExternalInput").ap()
    bb = nc.dram_tensor("bb", [P, 1], f32, kind="ExternalInput").ap()
    out = nc.dram_tensor("out", [P, TILES * (S + 1)], f32,
                         kind="ExternalOutput").ap()

    with tile.TileContext(nc) as tc:
        with (
            tc.tile_pool(name="const", bufs=1) as cpool,
            tc.tile_pool(name="hcT", bufs=2) as hcT_pool,
            tc.tile_pool(name="hpl", bufs=2) as hpl_pool,
            tc.tile_pool(name="g", bufs=6) as g_pool,
            tc.tile_pool(name="v", bufs=4) as v_pool,
            tc.tile_pool(name="prod", bufs=4) as prod_pool,
            tc.tile_pool(name="psum", bufs=4, space="PSUM") as psum_pool,
        ):
            if not cfg.host_gather:
                # All gather indices resident:
                # idx_sb[p, t*S+s] = sample[s, t*128+p]. Loaded FIRST so the
                # gather stream (the critical path) starts as early as possible.
                idx_sb = cpool.tile([P, TILES * S], mybir.dt.int32)
                nc.sync.dma_start(out=idx_sb[:], in_=idx[:])
            # W.T resident: free layout (c, d) — chunk c covers contraction
            # rows c*128..c*128+127.
            wt_sb = cpool.tile([P, KC * D], bf16)
            nc.sync.dma_start(
                out=wt_sb[:].rearrange("p (c d) -> p c d", c=KC),
                in_=wt.rearrange("(c p) d -> p c d", p=P))
            b_sb = cpool.tile([P, 1], f32)
            nc.sync.dma_start(out=b_sb[:], in_=bb[:])
            sc_acc = cpool.tile([P, TILES * (S + 1)], f32)
            junk = cpool.tile([P, D], bf16)    # discarded POOL-STT elementwise out
            junk2 = cpool.tile([P, D], bf16)   # discarded DVE-STT elementwise out
            dump = cpool.tile([P, D], bf16)    # discarded ACT elementwise out

            for t0 in range(0, TILES, cfg.super_tile):
                st = min(cfg.super_tile, TILES - t0)
                # hcT block [D, st*128] -> SBUF free layout (c, n_local)
                hcT_sb = hcT_pool.tile([P, KC * cfg.super_tile * P], bf16,
                                       tag="hcT")
                nc.sync.dma_start(
                    out=hcT_sb[:, : KC * st * P].rearrange(
                        "p (c n) -> p c n", c=KC),
                    in_=hcT[:, t0 * P:(t0 + st) * P].rearrange(
                        "(c p) n -> p c n", p=P),
                )
                # hpl block: rows t0*128..(t0+st)*128 -> [p, j, d]
                hpl_sb = hpl_pool.tile([P, cfg.super_tile * D], bf16, tag="hpl")
                nc.sync.dma_start(
                    out=hpl_sb[:, : st * D].rearrange("p (j d) -> p j d", j=st),
                    in_=hpl[t0 * P:(t0 + st) * P, :].rearrange(
                        "(j p) d -> p j d", p=P),
                )
                if cfg.host_gather:
                    # pre-gathered rows stream in as one sequential block
                    g_st = g_pool.tile([P, cfg.super_tile * S * D], bf16,
                                       tag="g")
                    nc.sync.dma_start(
                        out=g_st[:, : st * S * D].rearrange(
                            "p (j x) -> p j x", j=st),
                        in_=gd[t0 * P:(t0 + st) * P, :].rearrange(
                            "(j p) x -> p j x", p=P),
                    )
                for j in range(st):
                    t = t0 + j
                    if cfg.host_gather:
                        g_base, g_off = g_st, j * S * D
                    else:
                        # Gather the S sampled rows per node:
                        # g_sb[p, s*D:(s+1)*D] = hc[idx_sb[p, t*S+s], :]
                        # (HW indirect DMA honors one index per partition,
                        # so one call per s — multi-column calls return
                        # garbage on silicon.)
                        g_tile = g_pool.tile([P, S * D], bf16, tag="g")
                        for s in range(S):
                            gi = nc.gpsimd.indirect_dma_start(
                                out=g_tile[:, s * D:(s + 1) * D],
                                out_offset=None,
                                in_=hc[:],
                                in_offset=bass.IndirectOffsetOnAxis(
                                    ap=idx_sb[:, t * S + s:t * S + s + 1],
                                    axis=0),
                            )
                            q = s % cfg.nq
                            if q != 0:
                                gi.ins.queue = f"qPoolDynamic{q}"
                        g_base, g_off = g_tile, 0
                    # v = hc0_tile @ W.T via 4 accumulating matmuls (bf16)
                    v_ps = psum_pool.tile([P, D], f32, space="PSUM", tag="v_ps")
                    for c in range(KC):
                        off = (c * st + j) * P
                        nc.tensor.matmul(
                            out=v_ps[:],
                            lhsT=hcT_sb[:, off:off + P],
                            rhs=wt_sb[:, c * D:(c + 1) * D],
                            start=(c == 0),
                            stop=(c == KC - 1),
                        )
                    # v to SBUF as bf16 so DVE ops run in 2x-eligible mode
                    v_sb = v_pool.tile([P, D], bf16, tag="v")
                    nc.scalar.copy(v_sb[:], v_ps[:])

                    col = t * (S + 1)
                    hpl_t = hpl_sb[:, j * D:(j + 1) * D]

                    def x_row(s):
                        if s == 0:
                            return hpl_t
                        return g_base[:, g_off + (s - 1) * D:g_off + s * D]

                    for row in range(S + 1):
                        if row < cfg.pool_rows:
                            # fused mult+reduce on GpSimd (only viable when
                            # host_gather frees the Q7 from descriptor gen)
                            nc.gpsimd.scalar_tensor_tensor(
                                out=junk[:],
                                in0=x_row(row),
                                scalar=1.0,
                                in1=v_sb[:],
                                op0=mybir.AluOpType.mult,
                                op1=mybir.AluOpType.mult,
                                accum_out=sc_acc[:, col + row:col + row + 1],
                            )
                        elif row < cfg.pool_rows + cfg.stt_rows:
                            # fused mult+reduce on DVE: accum = sum(in0*in1)
                            nc.vector.scalar_tensor_tensor(
                                out=junk2[:],
                                in0=x_row(row),
                                scalar=1.0,
                                in1=v_sb[:],
                                op0=mybir.AluOpType.mult,
                                op1=mybir.AluOpType.mult,
                                accum_out=sc_acc[:, col + row:col + row + 1],
                            )
                        else:
                            # bf16 2x mult on DVE + ScalarE accum reduce
                            prod = prod_pool.tile([P, D], bf16, tag="prod")
                            nc.vector.tensor_mul(prod[:], x_row(row), v_sb[:])
                            nc.scalar.activation(
                                dump[:], prod[:],
                                mybir.ActivationFunctionType.Copy,
                                accum_out=sc_acc[:, col + row:col + row + 1],
                            )
            # bias epilogue (no accum-init on the fused ops)
            nc.vector.tensor_scalar_add(sc_acc[:], sc_acc[:], b_sb[:, :1])
            nc.sync.dma_start(out=out[:], in_=sc_acc[:])
    nc.compile()
    return nc


def build_nc_b(cfg: Cfg):
    """Layout-B kernel: d on partitions, nodes on the free axis.

    Host sends all five x-streams (h_pl row + 4 gathered rows) pre-transposed
    as one interleaved tensor xb[(u p), (c r f)]. Per super-block u of
    F=512 nodes:
      vT[c][p, f] = sum_e W[d,e] hcT[e,n]    (16 accumulating PE matmuls)
      prod = x * vT (broadcast over r)       (20 DVE bf16 2x mults)
      sc[r, f]  += sum_p prod[c][r][p, f]    (20 PE one-hot matmuls -> [5, F])
    so the per-node dot reductions ride the PE (128 elem/cycle) instead of
    DVE/ACT (1/cycle).
    """
    D, S, KC, F = cfg.d, cfg.s, cfg.kc, cfg.fsz
    R = S + 1
    bf16 = mybir.dt.bfloat16
    f32 = mybir.dt.float32

    nc = bacc.Bacc("TRN2", target_bir_lowering=False, debug=False)
    xb = nc.dram_tensor("xb", [cfg.n_super * P, KC * R * F], bf16,
                        kind="ExternalInput").ap()
    hcT = nc.dram_tensor("hcT", [D, cfg.npad], bf16, kind="ExternalInput").ap()
    wt = nc.dram_tensor("wt", [D, D], bf16, kind="ExternalInput").ap()
    oneh = nc.dram_tensor("oneh", [P, R * R], bf16, kind="ExternalInput").ap()
    bb = nc.dram_tensor("bb", [P, 1], f32, kind="ExternalInput").ap()
    out = nc.dram_tensor("out", [R, cfg.npad], f32, kind="ExternalOutput").ap()

    with tile.TileContext(nc) as tc:
        with (
            tc.tile_pool(name="const", bufs=1) as cpool,
            tc.tile_pool(name="hcT", bufs=2) as hcT_pool,
            tc.tile_pool(name="xb", bufs=2) as xb_pool,
            tc.tile_pool(name="vt", bufs=2) as vt_pool,
            tc.tile_pool(name="prod", bufs=2) as prod_pool,
            tc.tile_pool(name="psv", bufs=4, space="PSUM") as psv_pool,
            tc.tile_pool(name="pssc", bufs=2, space="PSUM") as pssc_pool,
        ):
            wt_sb = cpool.tile([P, KC * D], bf16)
            nc.sync.dma_start(
                out=wt_sb[:].rearrange("p (c d) -> p c d", c=KC),
                in_=wt.rearrange("(c p) d -> p c d", p=P))
            oneh_sb = cpool.tile([P, R * R], bf16)
            nc.sync.dma_start(out=oneh_sb[:], in_=oneh[:])
            b_sb = cpool.tile([P, 1], f32)
            nc.sync.dma_start(out=b_sb[:], in_=bb[:])
            sc_acc = cpool.tile([R, cfg.npad], f32)

            for u in range(cfg.n_super):
                hcT_sb = hcT_pool.tile([P, KC * F], bf16, tag="hcT")
                nc.sync.dma_start(
                    out=hcT_sb[:].rearrange("p (c n) -> p c n", c=KC),
                    in_=hcT[:, u * F:(u + 1) * F].rearrange(
                        "(c p) n -> p c n", p=P),
                )
                x_sb = xb_pool.tile([P, KC * R * F], bf16, tag="xb")
                nc.sync.dma_start(out=x_sb[:], in_=xb[u * P:(u + 1) * P, :])

                # vT chunks: vT[dc][p, f] with d = dc*128 + p
                vt_sb = vt_pool.tile([P, KC * F], bf16, tag="vt")
                for dc in range(KC):
                    vps = psv_pool.tile([P, F], f32, space="PSUM", tag="vps")
                    for ec in range(KC):
                        nc.tensor.matmul(
                            out=vps[:],
                            lhsT=wt_sb[:, ec * D + dc * P:
                                       ec * D + (dc + 1) * P],
                            rhs=hcT_sb[:, ec * F:(ec + 1) * F],
                            start=(ec == 0),
                            stop=(ec == KC - 1),
                        )
                    nc.scalar.copy(vt_sb[:, dc * F:(dc + 1) * F], vps[:])

                # products (DVE bf16 2x), layout [c][r][f]
                prod_sb = prod_pool.tile([P, KC * R * F], bf16, tag="prod")
                for dc in range(KC):
                    for r in range(R):
                        off = (dc * R + r) * F
                        nc.vector.tensor_mul(
                            prod_sb[:, off:off + F],
                            x_sb[:, off:off + F],
                            vt_sb[:, dc * F:(dc + 1) * F])

                # partition-axis reduce on PE: one-hot lhsT routes row r's
                # sum into PSUM partition r; all 20 matmuls accumulate.
                scps = pssc_pool.tile([R, F], f32, space="PSUM", tag="scps")
                nmm = R * KC
                for i in range(nmm):
                    r, dc = divmod(i, KC)
                    off = (dc * R + r) * F
                    nc.tensor.matmul(
                        out=scps[:],
                        lhsT=oneh_sb[:, r * R:(r + 1) * R],
                        rhs=prod_sb[:, off:off + F],
                        start=(i == 0),
                        stop=(i == nmm - 1),
                    )
                nc.scalar.copy(sc_acc[:, u * F:(u + 1) * F], scps[:])

            nc.vector.tensor_scalar_add(sc_acc[:], sc_acc[:], b_sb[0:R, :1])
            nc.sync.dma_start(out=out[:], in_=sc_acc[:])
    nc.compile()
    return nc


def make_in_maps(cfg: Cfg, h_c, h_pl, sample_list, W, b):
    """Host-side sharding: full inputs -> per-core input dicts (bf16 casts)."""
    D, S = cfg.d, cfg.s
    hc0 = np.asarray(h_c, np.float32)[0]
    hpl0 = np.asarray(h_pl, np.float32)[0]
    smp = np.asarray(sample_list)
    W0 = np.asarray(W, np.float32)[0]
    bval = float(np.asarray(b, np.float32).reshape(-1)[0])

    hc_bf = np.ascontiguousarray(hc0.astype(BF16))     # gather table [N, D]
    hcT_bf = np.ascontiguousarray(hc0.T.astype(BF16))  # [D, N]
    wt_bf = np.ascontiguousarray(W0.T.astype(BF16))    # wt[e, d] = W[d, e]
    b_bcast = np.full((P, 1), bval, np.float32)

    in_maps = []
    for c in range(cfg.n_cores):
        lo = c * cfg.nodes_per_core
        hi = lo + cfg.nodes_per_core
        hcT_s = np.zeros((D, cfg.npad), BF16)
        hcT_s[:, : cfg.nodes_per_core] = hcT_bf[:, lo:hi]
        hpl_s = np.zeros((cfg.npad, D), BF16)
        hpl_s[: cfg.nodes_per_core] = hpl0[lo:hi].astype(BF16)
        if cfg.layout_b:
            R = S + 1
            X = np.zeros((R, cfg.npad, D), BF16)
            X[0, : cfg.nodes_per_core] = hpl0[lo:hi].astype(BF16)
            X[1:, : cfg.nodes_per_core] = hc_bf[smp[:, lo:hi]]
            xb = np.ascontiguousarray(
                X.reshape(R, cfg.n_super, cfg.fsz, cfg.kc, P)
                .transpose(1, 4, 3, 0, 2)
                .reshape(cfg.n_super * P, cfg.kc * R * cfg.fsz))
            oneh = np.zeros((P, R * R), BF16)
            for r in range(R):
                oneh[:, r * R + r] = 1.0
            in_maps.append({
                "xb": xb, "hcT": hcT_s, "wt": wt_bf, "oneh": oneh,
                "bb": b_bcast,
            })
            continue
        im = {"hcT": hcT_s, "hpl": hpl_s, "wt": wt_bf, "bb": b_bcast}
        if cfg.host_gather:
            # g[node, s*D:(s+1)*D] = hc_bf[sample[s, node]] (node tile-major)
            g_s = np.zeros((cfg.npad, S, D), BF16)
            g_s[: cfg.nodes_per_core] = hc_bf[smp[:, lo:hi]].transpose(1, 0, 2)
            im["g"] = np.ascontiguousarray(g_s.reshape(cfg.npad, S * D))
        else:
            idx_s = np.zeros((S, cfg.npad), np.int64)
            idx_s[:, : cfg.nodes_per_core] = smp[:, lo:hi]
            im["idx"] = np.ascontiguousarray(
                idx_s.reshape(S, cfg.tiles, P).transpose(2, 1, 0)
                .astype(np.int32).reshape(P, cfg.tiles * S))
            im["hc"] = hc_bf
        in_maps.append(im)
    return in_maps


def assemble_output(cfg: Cfg, outs):
    """Per-core 'out' arrays -> full logits [1, N + 2*S*N]."""
    S = cfg.s
    n = cfg.nodes_per_core * cfg.n_cores
    sc1 = np.empty((n,), np.float32)
    sc2 = np.empty((S, n), np.float32)
    for c in range(cfg.n_cores):
        if cfg.layout_b:
            o = outs[c][:, : cfg.nodes_per_core]   # [S+1, nodes]
        else:
            o = (outs[c].reshape(P, cfg.tiles, S + 1).transpose(2, 1, 0)
                 .reshape(S + 1, cfg.npad)[:, : cfg.nodes_per_core])
        lo = c * cfg.nodes_per_core
        sc1[lo:lo + cfg.nodes_per_core] = o[0]
        sc2[:, lo:lo + cfg.nodes_per_core] = o[1:]
    flat = sc2.reshape(-1)
    return np.concatenate([sc1, flat, flat])[None, :].astype(np.float32)


_NC_CACHE = {}


def _get_nc(cfg: Cfg):
    key = (cfg.n_table, cfg.nodes_per_core, cfg.d, cfg.s, cfg.super_tile,
           cfg.stt_rows, cfg.nq, cfg.gather_cols, cfg.host_gather,
           cfg.pool_rows, cfg.layout_b)
    if key not in _NC_CACHE:
        _NC_CACHE[key] = build_nc_b(cfg) if cfg.layout_b else build_nc(cfg)
    return _NC_CACHE[key]


def run_on_hw(cfg: Cfg, inputs, trace=False, trace_kwargs={}):
    nc = _get_nc(cfg)
    in_maps = make_in_maps(cfg, **inputs)
    res = run_bass_kernel_spmd(nc, in_maps, core_ids=list(range(cfg.n_cores)),
                               trace=trace, trace_kwargs=trace_kwargs)
    out = assemble_output(cfg, [r["out"] for r in res.results])
    return out, res


def kernel(h_c, h_pl, sample_list, W, b):
    inputs = dict(h_c=h_c, h_pl=h_pl, sample_list=sample_list, W=W, b=b)
    out, _ = run_on_hw(FULL, inputs, trace=False)
    return out
